# revision 6
# baseline (speedup 1.0000x reference)
"""Trainium2 Bass kernel for nn_ConvolveTensor3body.

Pipeline per core (2500 nodes, data-parallel over 8 cores):
  X [c=128,(n,i)] --PE mix (lhsT=X-group)--> h[(n8,i),(m,o)] --ACT silu-->
  per-k eigendecomp of W3J (exact, total rank 128): out_k = sum_r C[r,k] (U x)_r (U y)_r
  P/Q projections (PE, block-diag-8 weights), E = P*Q (DVE), Esum (PE) -> out2, out3
  PE transposes back to [c,(n,k)], staged interleave, DMA out [N,C,16,3].
All matmuls in float32r (full rate at N>=256).
"""
import numpy as np
from math import factorial, sqrt

import concourse.bass as bass
from concourse import mybir
from concourse.bass_utils import run_bass_kernel_spmd

NODES = 20000
NCH = 128
DIM = 16
NCORES = 8
NN = NODES // NCORES          # 2500 nodes per core
ST = 32                       # nodes per super-tile (4 groups of 8)
NST = (NN + ST - 1) // ST     # 79 super-tiles (last partial: 4 nodes)
NCHUNK = 8                    # 16-row chunks per contraction (R=128)

F32 = mybir.dt.float32
F32R = mybir.dt.float32r
AF = mybir.ActivationFunctionType


# ---------------- W3J + eigendecomposition (host-side, exact) ----------------

def _f(n):
    return factorial(round(n))


def _su2_cg(j1, m1, j2, m2, j3, m3):
    if m3 != m1 + m2:
        return 0.0
    vmin = int(max(-j1 + j2 + m3, -j1 + m1, 0))
    vmax = int(min(j2 + j3 + m1, j3 - j1 + j2, j3 + m3))
    if vmax < vmin:
        return 0.0
    C = sqrt((2 * j3 + 1)
             * _f(j3 + j1 - j2) * _f(j3 - j1 + j2) * _f(j1 + j2 - j3)
             * _f(j3 + m3) * _f(j3 - m3)
             / (_f(j1 + j2 + j3 + 1) * _f(j1 - m1) * _f(j1 + m1)
                * _f(j2 - m2) * _f(j2 + m2)))
    S = 0.0
    for v in range(vmin, vmax + 1):
        S += (-1.0) ** (v + j2 + m2) * _f(j2 + j3 + m1 - v) * _f(j1 - m1 + v) / (
            _f(v) * _f(j3 - j1 + j2 - v) * _f(j3 + m3 - v) * _f(v + j1 - j2 - m3))
    return C * S


def _q_real_to_complex(l):
    q = np.zeros((2 * l + 1, 2 * l + 1), dtype=np.complex128)
    for m in range(-l, 0):
        q[l + m, l + abs(m)] = 1.0 / np.sqrt(2.0)
        q[l + m, l - abs(m)] = -1j / np.sqrt(2.0)
    q[l, l] = 1.0
    for m in range(1, l + 1):
        q[l + m, l + abs(m)] = (-1) ** m / np.sqrt(2.0)
        q[l + m, l - abs(m)] = 1j * (-1) ** m / np.sqrt(2.0)
    return (-1j) ** l * q


def _real_w3j(l1, l2, l3):
    C = np.zeros((2 * l1 + 1, 2 * l2 + 1, 2 * l3 + 1), dtype=np.complex128)
    for i, m1 in enumerate(range(-l1, l1 + 1)):
        for j, m2 in enumerate(range(-l2, l2 + 1)):
            for k, m3 in enumerate(range(-l3, l3 + 1)):
                C[i, j, k] = _su2_cg(l1, m1, l2, m2, l3, m3)
    Cr = np.real(np.einsum('il,jm,kn,lmn->ijk',
                           _q_real_to_complex(l1), _q_real_to_complex(l2),
                           np.conj(_q_real_to_complex(l3)), C))
    n = np.linalg.norm(Cr)
    return Cr / n if n > 0 else Cr


def _build_w3j(lmax=3):
    D = (lmax + 1) ** 2
    W = np.zeros((D, D, D))
    for l1 in range(lmax + 1):
        for l2 in range(lmax + 1):
            for l3 in range(abs(l1 - l2), min(l1 + l2, lmax) + 1):
                if (l1 + l2 + l3) % 2:
                    continue
                W[l1 * l1:(l1 + 1) ** 2, l2 * l2:(l2 + 1) ** 2,
                  l3 * l3:(l3 + 1) ** 2] = _real_w3j(l1, l2, l3)
    return W


def _build_eig_factors():
    """U [128,16], C [128,16]: out_k = sum_r C[r,k] (U x)_r (U y)_r exactly."""
    W = _build_w3j(3)
    Urows, Crows = [], []
    for k in range(DIM):
        ev, V = np.linalg.eigh(W[:, :, k])
        for lam, v in zip(ev, V.T):
            if abs(lam) > 1e-10:
                Urows.append(v)
                c = np.zeros(DIM)
                c[k] = lam
                Crows.append(c)
    U = np.array(Urows)
    C = np.array(Crows)
    assert U.shape[0] == 128, U.shape
    return U.astype(np.float64), C.astype(np.float64)


def _build_consts():
    """One [128, 2560] f32 tile: wAll | U_bd x8 | C_bd x8 | identity."""
    U, C = _build_eig_factors()
    cols = []
    # placeholder for wAll (filled at call time with actual W0/W1/W2)
    bd_u = np.zeros((NCHUNK, 128, 128), dtype=np.float32)
    bd_c = np.zeros((NCHUNK, 128, 128), dtype=np.float32)
    for j in range(NCHUNK):
        for g in range(8):
            # P[(g,r), col] = sum_i U_bd[(g,i),(g,r)] * rhs[(g,i), col]
            bd_u[j, g * 16:(g + 1) * 16, g * 16:(g + 1) * 16] = U[16 * j:16 * j + 16, :].T
            # out2[(g,k), col] += sum_r C_bd[(g,r),(g,k)] * E[(g,r), col]
            bd_c[j, g * 16:(g + 1) * 16, g * 16:(g + 1) * 16] = C[16 * j:16 * j + 16, :]
    ident = np.eye(128, dtype=np.float32)
    return bd_u, bd_c, ident


_BD_U, _BD_C, _IDENT = None, None, None


def _consts_array(W0, W1, W2):
    global _BD_U, _BD_C, _IDENT
    if _BD_U is None:
        _BD_U, _BD_C, _IDENT = _build_consts()
    scale = 1.0 / np.sqrt(np.float32(NCH))
    wall = np.concatenate([W0, W1, W2], axis=1).astype(np.float32) * scale  # [128,384]
    cols = [wall]
    cols.extend(_BD_U[j] for j in range(NCHUNK))
    cols.extend(_BD_C[j] for j in range(NCHUNK))
    cols.append(_IDENT)
    return np.ascontiguousarray(np.concatenate(cols, axis=1), dtype=np.float32)


NCONST = 384 + 128 * NCHUNK * 2 + 128  # 2560


# ---------------- kernel program ----------------

def _build_program(nn):
    nst = (nn + ST - 1) // ST
    nc = bass.Bass()
    x = nc.dram_tensor("xin", [nn, NCH, DIM], F32, kind="ExternalInput")
    cst = nc.dram_tensor("consts", [128, NCONST], F32, kind="ExternalInput")
    y = nc.dram_tensor("yout", [nn, NCH, DIM, 3], F32, kind="ExternalOutput")

    x_r = x.rearrange("n c i -> c n i")          # [128, nn, 16]
    y_r = y.rearrange("n c k t -> c n (k t)")    # [128, nn, 48]

    ctxs = []

    def mk_sbuf(shape, dtype=F32):
        cm = nc.sbuf_tensor(shape, dtype)
        t = cm.__enter__()
        ctxs.append(cm)
        return t

    def mk_psum(shape):
        cm = nc.psum_tensor(shape, F32)
        t = cm.__enter__()
        ctxs.append(cm)
        return t

    def mk_sem():
        cm = nc.semaphore()
        s = cm.__enter__()
        ctxs.append(cm)
        return s

    consts = mk_sbuf([128, NCONST], F32R)
    xT = mk_sbuf([128, 2, 512], F32R)          # [c, buf, (n32 i16)]
    hs = mk_sbuf([128, 2, 4, 3, 128], F32R)    # [(n8 i), buf, g, m, o]
    Qs = mk_sbuf([128, 2, 512], F32R)
    Es = mk_sbuf([128, 2, 512], F32R)
    o2s = mk_sbuf([128, 2, 512], F32R)
    o3s = mk_sbuf([128, 2, 512], F32R)
    stg = mk_sbuf([128, 2, ST * 48])     # [c, buf, (n k t)]

    # PSUM banks
    mixg = [mk_psum([128, 512]) for _ in range(4)]      # b0..b3 (b0=out2 acc, b1=out3 acc reuse)
    Pb = [mk_psum([128, 512]) for _ in range(2)]
    Qb = [mk_psum([128, 512]) for _ in range(2)]

    wAll = consts[:, 0:384]
    U_bd = [consts[:, 384 + 128 * j: 384 + 128 * (j + 1)] for j in range(NCHUNK)]
    C_bd = [consts[:, 1408 + 128 * j: 1408 + 128 * (j + 1)] for j in range(NCHUNK)]
    ident = consts[:, 2432:2560]

    s_cin = mk_sem()
    s_in = mk_sem()
    s_mix = mk_sem()
    s_silu = mk_sem()
    s_pq = mk_sem()
    s_qe = mk_sem()
    s_e = mk_sem()
    s_o2 = mk_sem()
    s_o2e = mk_sem()
    s_o3 = mk_sem()
    s_o3e = mk_sem()
    s_tp = mk_sem()
    s_stA = mk_sem()
    s_stD = mk_sem()
    s_dout = mk_sem()

    NCH2 = 2 * NCHUNK  # chunks per super-tile

    r32 = lambda ap: ap

    with nc.Block() as block:

        @block.gpsimd
        def _(gps):
            gps.dma_start(consts[:], cst[:]).then_inc(s_cin, 16)
            for t in range(nst):
                n0 = t * ST
                nnn = min(ST, nn - n0)
                if t >= 2:
                    gps.wait_ge(s_mix, t - 1)
                gps.dma_start(
                    xT[:, t % 2, 0:nnn * 16].rearrange("c (n i) -> c n i", i=16),
                    x_r[:, n0:n0 + nnn, :],
                ).then_inc(s_in, 16)

        @block.sync
        def _(sync):
            for t in range(nst):
                n0 = t * ST
                nnn = min(ST, nn - n0)
                sync.wait_ge(s_stA, t + 1)
                sync.wait_ge(s_stD, 2 * (t + 1))
                sync.dma_start(
                    y_r[:, n0:n0 + nnn, :],
                    stg[:, t % 2, 0:nnn * 48].rearrange("c (n kt) -> c n kt", kt=48),
                ).then_inc(s_dout, 16)

        @block.tensor
        def _(pe):
            pe.wait_ge(s_cin, 16)
            for t in range(nst):
                C0 = t * NCH2
                # WAR on accum/transpose banks from previous super-tile
                if t >= 1:
                    pe.wait_ge(s_o2e, t)
                    pe.wait_ge(s_o3e, t)
                    pe.wait_ge(s_stA, t)
                    pe.wait_ge(s_stD, 2 * t)
                pe.wait_ge(s_in, 16 * (t + 1))
                for g in range(4):
                    mm = nc.tensor.matmul(
                        mixg[g][:, 0:384],
                        r32(xT[:, t % 2, g * 128:(g + 1) * 128]),
                        r32(wAll),
                        start=True, stop=True,
                    )
                    if g == 3:
                        mm.then_inc(s_mix, 1)

                def chunks(base, xrhs, yrhs, accbank, s_done):
                    # xrhs/yrhs: [128, 512] APs; accbank: psum
                    for j in range(NCHUNK):
                        Cg = base + j
                        p = Cg % 2
                        if Cg >= 2:
                            pe.wait_ge(s_e, Cg - 1)
                            pe.wait_ge(s_qe, Cg - 1)
                        nc.tensor.matmul(Pb[p][:, :], r32(U_bd[j]), xrhs,
                                         start=True, stop=True)
                        nc.tensor.matmul(Qb[p][:, :], r32(U_bd[j]), yrhs,
                                         start=True, stop=True).then_inc(s_pq, 1)
                        if j >= 1:
                            pe.wait_ge(s_e, base + j)
                            nc.tensor.matmul(accbank[:, 0:512], r32(C_bd[j - 1]),
                                             r32(Es[:, (base + j - 1) % 2, :]),
                                             start=(j == 1), stop=False)
                    pe.wait_ge(s_e, base + NCHUNK)
                    nc.tensor.matmul(accbank[:, 0:512], r32(C_bd[NCHUNK - 1]),
                                     r32(Es[:, (base + NCHUNK - 1) % 2, :]),
                                     start=False, stop=True).then_inc(s_done, 1)

                pe.wait_ge(s_silu, 4 * (t + 1))
                h0 = r32(hs[:, t % 2, :, 0, :])
                h1 = r32(hs[:, t % 2, :, 1, :])
                h2 = r32(hs[:, t % 2, :, 2, :])
                chunks(C0, h0, h1, mixg[0], s_o2)
                pe.wait_ge(s_o2e, t + 1)
                chunks(C0 + NCHUNK, r32(o2s[:, t % 2, :]), h2, mixg[1], s_o3)
                # back-transposes into Qb[0] (node1), Qb[1] (node2), Pb[1] (node3)
                pe.wait_ge(s_o3e, t + 1)
                pe.wait_ge(s_qe, C0 + NCH2)
                for g in range(4):
                    mm = nc.tensor.transpose(
                        Qb[0][:, g * 128:(g + 1) * 128].bitcast(F32R),
                        hs[:, t % 2, g, 0, :],
                        ident)
                    if g == 3:
                        mm.then_inc(s_tp, 1)
                for g in range(4):
                    mm = nc.tensor.transpose(
                        Qb[1][:, g * 128:(g + 1) * 128].bitcast(F32R),
                        o2s[:, t % 2, g * 128:(g + 1) * 128],
                        ident)
                    if g == 3:
                        mm.then_inc(s_tp, 1)
                for g in range(4):
                    mm = nc.tensor.transpose(
                        Pb[1][:, g * 128:(g + 1) * 128].bitcast(F32R),
                        o3s[:, t % 2, g * 128:(g + 1) * 128],
                        ident)
                    if g == 3:
                        mm.then_inc(s_tp, 1)

        @block.scalar
        def _(act):
            for t in range(nst):
                C0 = t * NCH2
                act.wait_ge(s_mix, t + 1)
                if t >= 1:
                    act.wait_ge(s_tp, 3 * t)   # hs[t-1] fully consumed
                for g in range(4):
                    nc.scalar.activation(
                        hs[:, t % 2, g, :, :].rearrange("p m o -> p (m o)"),
                        mixg[g][:, 0:384], AF.Silu).then_inc(s_silu, 1)

                def qevacs(base):
                    for j in range(NCHUNK):
                        Cg = base + j
                        act.wait_ge(s_pq, Cg + 1)
                        if Cg >= 2:
                            act.wait_ge(s_e, Cg - 1)
                        nc.scalar.activation(Qs[:, Cg % 2, :], Qb[Cg % 2][:, :],
                                             AF.Copy).then_inc(s_qe, 1)

                qevacs(C0)
                act.wait_ge(s_o2, t + 1)
                nc.scalar.activation(o2s[:, t % 2, :], mixg[0][:, 0:512],
                                     AF.Copy).then_inc(s_o2e, 1)
                qevacs(C0 + NCHUNK)
                act.wait_ge(s_o3, t + 1)
                nc.scalar.activation(o3s[:, t % 2, :], mixg[1][:, 0:512],
                                     AF.Copy).then_inc(s_o3e, 1)
                # staging: node2 (slot 1) from Qb[1] transpose
                act.wait_ge(s_tp, 3 * t + 2)
                if t >= 2:
                    act.wait_ge(s_dout, 16 * (t - 1))
                nc.scalar.activation(
                    stg[:, t % 2, :].rearrange("c (nk t3) -> c nk t3", t3=3)[:, :, 1],
                    Qb[1][:, :], AF.Copy).then_inc(s_stA, 1)

        @block.vector
        def _(dve):
            for t in range(nst):
                C0 = t * NCH2
                for j in range(NCH2):
                    Cg = C0 + j
                    dve.wait_ge(s_qe, Cg + 1)
                    nc.vector.tensor_mul(Es[:, Cg % 2, :], Pb[Cg % 2][:, :],
                                         Qs[:, Cg % 2, :]).then_inc(s_e, 1)
                # staging node1 (slot 0) from Qb[0], node3 (slot 2) from Pb[1]
                dve.wait_ge(s_tp, 3 * t + 1)
                if t >= 2:
                    dve.wait_ge(s_dout, 16 * (t - 1))
                nc.vector.tensor_copy(
                    stg[:, t % 2, :].rearrange("c (nk t3) -> c nk t3", t3=3)[:, :, 0],
                    Qb[0][:, :]).then_inc(s_stD, 1)
                dve.wait_ge(s_tp, 3 * t + 3)
                nc.vector.tensor_copy(
                    stg[:, t % 2, :].rearrange("c (nk t3) -> c nk t3", t3=3)[:, :, 2],
                    Pb[1][:, :]).then_inc(s_stD, 1)

    for cm in reversed(ctxs):
        cm.__exit__(None, None, None)
    return nc


# ---------------- public entry ----------------

def kernel(node_feature_i, W0, W1, W2):
    node_feature_i = np.ascontiguousarray(node_feature_i, dtype=np.float32)
    consts = _consts_array(np.asarray(W0, np.float32), np.asarray(W1, np.float32),
                           np.asarray(W2, np.float32))
    nc = _build_program(NN)
    in_maps = []
    for c in range(NCORES):
        in_maps.append({
            "xin": node_feature_i[c * NN:(c + 1) * NN],
            "consts": consts,
        })
    res = run_bass_kernel_spmd(nc, in_maps, core_ids=list(range(NCORES)))
    out = np.concatenate([r["yout"] for r in res.results], axis=0)
    return out
